# revision 1
# baseline (speedup 1.0000x reference)
"""Trainium2 Bass kernel for quantized int8 per-channel Conv2d.

Reference semantics (fp32):
  x_f = (x_int8 - 7) * 0.01                      # per-tensor dequant
  w_f = (w_int8 - zp[cout]) * scale[cout]        # per-channel dequant
  y   = round(conv2d_valid(x_f, w_f) + bias[cout])  -> int32

Exact-integer factorization used here:
  conv(x_f, w_f) = 0.01*scale[o] * S(o, p),  S = conv((x-7), (w-zp[o]))
(x-7) in [-135,120] and (w-zp) in [-137,137] are exact in bf16; products
accumulate exactly in fp32 PSUM (|S| << 2^24 for this data).  The final
affine + round happens in fp32 with the 1.5*2^23 magic-number trick,
which rounds half-to-even exactly like jnp.round.

Sharding: data-parallel over batch N=32 across 8 cores (4 images each);
weights/scales/zeropoints/bias replicated.
"""

import numpy as np

import concourse.bass as bass
import concourse.mybir as mybir
from concourse import bacc
from concourse.tile import TileContext
from concourse.bass_utils import run_bass_kernel_spmd

# Problem shapes (hardcoded per contract)
N, CIN, H, W = 32, 256, 56, 56
COUT, KH, KW = 256, 3, 3
HO, WO = H - KH + 1, W - KW + 1          # 54, 54
NCORES = 8
NPER = N // NCORES                        # images per core
HW = H * W                                # 3136
XPAD = HW + 4                             # pad: tap (2,2) of last chunk reads 2 past
CHUNK = 9 * WO                            # 486 = 9 output rows x 54 valid cols
NCHUNK = (HO * WO) // CHUNK               # 6
KT = (CIN // 128)                         # 2 cin tiles
MT = COUT // 128                          # 2 cout tiles
TAPS = KH * KW                            # 9
MAGIC = 12582912.0                        # 1.5 * 2**23  (fp32 RNE rounding trick)
B_CHUNK = 3                               # chunks per matmul weight-reuse block

_CACHE = {}


def _build_program():
    nc = bacc.Bacc("TRN2", target_bir_lowering=False, debug=False,
                   num_devices=NCORES)
    dt = mybir.dt

    x_d = nc.dram_tensor("x", [NPER, CIN, H, W], dt.int8, kind="ExternalInput")
    wt_d = nc.dram_tensor("wt", [TAPS, CIN, COUT], dt.int8, kind="ExternalInput")
    sc_d = nc.dram_tensor("scales", [COUT], dt.float32, kind="ExternalInput")
    zp_d = nc.dram_tensor("zp", [COUT], dt.int32, kind="ExternalInput")
    bi_d = nc.dram_tensor("bias", [COUT], dt.float32, kind="ExternalInput")
    out_d = nc.dram_tensor("out", [NPER, COUT, HO, WO], dt.int32,
                           kind="ExternalOutput")

    with TileContext(nc) as tc:
        with (
            tc.tile_pool(name="const", bufs=1) as cpool,
            tc.tile_pool(name="xin", bufs=2) as xpool,
            tc.tile_pool(name="xbf", bufs=2) as xbpool,
            tc.tile_pool(name="psum", bufs=2 * B_CHUNK, space="PSUM") as ppool,
            tc.tile_pool(name="tmp", bufs=4) as tpool,
            tc.tile_pool(name="outb", bufs=6) as opool,
        ):
            # ---- one-time constants ----
            # zeropoint row replicated to all 128 partitions via step-0 DMA
            zpb = cpool.tile([128, COUT], dt.int32)
            nc.sync.dma_start(out=zpb[:, :],
                              in_=zp_d[None, :].to_broadcast([128, COUT]))

            # PE warm-up: ~40 tiny matmuls fill the input-DMA wait right
            # after the boot barrier, flipping the HAM clock gate to 8/8
            # before the first real conv matmul issues.
            wupw = cpool.tile([1, 1], dt.bfloat16)
            nc.vector.memset(wupw[:, :], 1.0)
            wupx = cpool.tile([1, 128], dt.bfloat16)
            nc.vector.memset(wupx[:, :], 1.0)
            wups = ppool.tile([1, 128], dt.float32, name="wups", tag="wup",
                              bufs=1)
            for _ in range(40):
                nc.tensor.matmul(wups[:, :], wupw[:, :], wupx[:, :],
                                 start=True, stop=True)

            def load_image(n, pieces=1):
                # DMA + (x-7) bf16 conversion, split into `pieces` column
                # blocks per cin-tile so downstream matmuls (region-level
                # deps) can start before the whole image has landed.
                xi = xpool.tile([128, KT, XPAD], dt.int8, name="xi")
                xb = xbpool.tile([128, KT, XPAD], dt.bfloat16, name="xb")
                bnd = [0, 1680, HW] if pieces == 2 else [0, HW]
                for k in range(KT):
                    for p in range(len(bnd) - 1):
                        a, b = bnd[p], bnd[p + 1]
                        be = b if b < HW else XPAD  # convert pad cols too
                        nc.sync.dma_start(
                            out=xi[:, k, a:b],
                            in_=x_d[n, k * 128:(k + 1) * 128].rearrange(
                                "p h w -> p (h w)")[:, a:b])
                        # x' = x - 7, exact in bf16 (pad cols: finite garbage)
                        nc.vector.tensor_scalar(
                            xb[:, k, a:be], xi[:, k, a:be], -7.0,
                            None, mybir.AluOpType.add)
                return xb

            # ---- weights: int8 [tap, cin, cout] -> bf16 (w - zp) lhsT ----
            # Emission interleaves the first image's input load with the
            # weight DMA+subtracts in matmul consumption order (k-major),
            # so the first conv matmul fires as soon as tap (0,0) weights
            # and the first x columns have landed.  One weight DMA per
            # cin-tile (issue slots on the Sync queue cost ~620ns each).
            wi8 = cpool.tile([128, TAPS, KT, COUT], dt.int8)
            wb = cpool.tile([128, TAPS * KT, COUT], dt.bfloat16)

            xi0 = xpool.tile([128, KT, XPAD], dt.int8, name="xi")
            xb0 = xbpool.tile([128, KT, XPAD], dt.bfloat16, name="xb")
            XSPLIT = 1680  # covers chunk-block 0 reads (max 1626)

            def xdma0(k, a, b):
                nc.sync.dma_start(
                    out=xi0[:, k, a:b],
                    in_=x_d[0, k * 128:(k + 1) * 128].rearrange(
                        "p h w -> p (h w)")[:, a:b])

            def xconv0(k, a, b):
                nc.vector.tensor_scalar(xb0[:, k, a:b], xi0[:, k, a:b],
                                        -7.0, None, mybir.AluOpType.add)

            def wsub(k, t):
                nc.vector.tensor_tensor(
                    wb[:, t * KT + k, :], wi8[:, t, k, :], zpb[:, :],
                    mybir.AluOpType.subtract)

            xdma0(0, 0, XSPLIT)
            nc.sync.dma_start(
                out=wi8[:, 0:3, 0, :],
                in_=wt_d[0:3, 0:128, :].rearrange("t p o -> p t o"))
            nc.sync.dma_start(
                out=wi8[:, 3:TAPS, 0, :],
                in_=wt_d[3:TAPS, 0:128, :].rearrange("t p o -> p t o"))
            wsub(0, 0)
            xconv0(0, 0, XSPLIT)
            for t in range(1, TAPS):
                wsub(0, t)
            xdma0(0, XSPLIT, HW)
            xconv0(0, XSPLIT, XPAD)
            xdma0(1, 0, XSPLIT)
            xconv0(1, 0, XSPLIT)
            nc.sync.dma_start(
                out=wi8[:, :, 1, :],
                in_=wt_d[:, 128:256, :].rearrange("t p o -> p t o"))
            for t in range(TAPS):
                wsub(1, t)
            xdma0(1, XSPLIT, HW)
            xconv0(1, XSPLIT, XPAD)

            # combined output scale 0.01*scale[o] and bias, one column per m-tile
            sc2 = cpool.tile([128, MT], dt.float32)
            nc.sync.dma_start(out=sc2[:, :], in_=sc_d.rearrange("(m p) -> p m", p=128))
            nc.vector.tensor_scalar(sc2[:, :], sc2[:, :], 0.01, None,
                                    mybir.AluOpType.mult)
            bi2 = cpool.tile([128, MT], dt.float32)
            nc.sync.dma_start(out=bi2[:, :], in_=bi_d.rearrange("(m p) -> p m", p=128))

            # ---- per-image pipeline ----
            for n in range(NPER):
                xb = xb0 if n == 0 else load_image(n)

                for m in range(MT):
                    for cb in range(NCHUNK // B_CHUNK):
                        ps = [ppool.tile([128, CHUNK], dt.float32,
                                         name="ps", tag="ps")
                              for _ in range(B_CHUNK)]
                        # Final block runs chunk-major so per-chunk stops
                        # stagger and the tail epilogue overlaps the last
                        # matmuls (costs extra LDWEIGHTS, tail-only).
                        last_block = (n == NPER - 1 and m == MT - 1
                                      and cb == NCHUNK // B_CHUNK - 1)
                        def rhs_ap(k, c, dh, dw):
                            # 9 output rows x 54 valid cols of the shifted
                            # image: 2-level free AP (row stride 56) skips
                            # the 2 conv-overhang columns per row.
                            base = (9 * c + dh) * W + dw
                            return xb[:, k, base:base + 9 * W].rearrange(
                                "p (r w) -> p r w", w=W)[:, :, 0:WO]

                        if last_block:
                            for c0 in range(B_CHUNK):
                                c = cb * B_CHUNK + c0
                                first = True
                                for k in range(KT):
                                    for t in range(TAPS):
                                        dh, dw = t // KW, t % KW
                                        nc.tensor.matmul(
                                            ps[c0][:, :],
                                            wb[:, t * KT + k,
                                               m * 128:(m + 1) * 128],
                                            rhs_ap(k, c, dh, dw),
                                            start=first,
                                            stop=(k == KT - 1 and
                                                  t == TAPS - 1))
                                        first = False
                        else:
                            first = True
                            for k in range(KT):
                                for t in range(TAPS):
                                    dh, dw = t // KW, t % KW
                                    lhsT = wb[:, t * KT + k,
                                              m * 128:(m + 1) * 128]
                                    for c0 in range(B_CHUNK):
                                        c = cb * B_CHUNK + c0
                                        nc.tensor.matmul(
                                            ps[c0][:, :], lhsT,
                                            rhs_ap(k, c, dh, dw),
                                            start=first,
                                            stop=(k == KT - 1 and t == TAPS - 1))
                                    first = False
                        for c0 in range(B_CHUNK):
                            c = cb * B_CHUNK + c0
                            # y = 0.01*scale*S + bias   (fp32, per-partition)
                            tmp = tpool.tile([128, CHUNK], dt.float32)
                            nc.vector.tensor_scalar(
                                tmp[:, :], ps[c0][:, :],
                                sc2[:, m:m + 1], bi2[:, m:m + 1],
                                mybir.AluOpType.mult, mybir.AluOpType.add)
                            # round-to-nearest-even (psum already garbage-free)
                            t3 = tmp[:, :].rearrange("p (r w) -> p r w", w=WO)
                            ob = opool.tile([128, 9, WO], dt.int32)
                            nc.vector.tensor_scalar(
                                ob[:, :, :], t3[:, :, :], MAGIC, MAGIC,
                                mybir.AluOpType.add, mybir.AluOpType.subtract)
                            nc.sync.dma_start(
                                out=out_d[n, m * 128:(m + 1) * 128,
                                          9 * c:9 * (c + 1), :],
                                in_=ob[:, :, :])

    nc.compile()
    return nc


def kernel(**inputs) -> np.ndarray:
    x = np.ascontiguousarray(np.asarray(inputs["inputVec"], dtype=np.int8))
    w = np.asarray(inputs["weight"], dtype=np.int8)
    scales = np.ascontiguousarray(np.asarray(inputs["scales"], dtype=np.float32))
    zp = np.ascontiguousarray(np.asarray(inputs["zeropoints"], dtype=np.int32))
    bias = np.ascontiguousarray(np.asarray(inputs["bias"], dtype=np.float32))
    assert x.shape == (N, CIN, H, W) and w.shape == (COUT, CIN, KH, KW)

    # [cout, cin, kh, kw] -> [tap, cin, cout] so lhsT tiles DMA contiguously
    wt = np.ascontiguousarray(
        w.transpose(2, 3, 1, 0).reshape(TAPS, CIN, COUT))

    if "nc" not in _CACHE:
        _CACHE["nc"] = _build_program()
    nc = _CACHE["nc"]

    in_maps = [
        {"x": x[c * NPER:(c + 1) * NPER], "wt": wt, "scales": scales,
         "zp": zp, "bias": bias}
        for c in range(NCORES)
    ]
    res = run_bass_kernel_spmd(nc, in_maps, list(range(NCORES)))
    out = np.concatenate([res.results[c]["out"] for c in range(NCORES)], axis=0)
    return out



# revision 8
# speedup vs baseline: 1.2153x; 1.2153x over previous
"""Trainium2 Bass kernel for quantized int8 per-channel Conv2d.

Reference semantics (fp32):
  x_f = (x_int8 - 7) * 0.01                      # per-tensor dequant
  w_f = (w_int8 - zp[cout]) * scale[cout]        # per-channel dequant
  y   = round(conv2d_valid(x_f, w_f) + bias[cout])  -> int32

Winograd F(2,3) along W (direct conv along H), exact in fp16:
  Per 2 output cols j..j+1 and tap row dh, with g = w - zp (host-side):
    V0 = x0-x2, V1 = x1+x2, V2 = x2-x1, V3 = x1-x3   (ints <= 270: fp16-exact)
    U0 = g0, U1 = (g0+g1+g2)/2, U2 = (g0-g1+g2)/2, U3 = g2  (halves: fp16-exact)
    y0 = sum m0+m1+m2,  y1 = sum m1-m2-m3,  m_u = V_u . U_u  (over cin, dh)
  12 column-streams per output pixel per m-tile instead of 18 -> 0.667x PE time.
  The x-7 offset cancels in V0/V2/V3 and contributes a per-cout constant via
  V1 (since 2*sum U1 = sum g exactly), folded into bias on the host:
    bias_f = bias - 0.07*scale*sum(w - zp).
  Products are half-integers < 2^17, accumulated exactly in fp32 PSUM; final
  affine + magic-number round matches jnp.round to ~1e-5 rel (half-ULP ties).

Sharding: data-parallel over batch N=32 across 8 cores (4 images each);
weights/scales/bias replicated.
"""

import numpy as np

import concourse.bass as bass
import concourse.mybir as mybir
from concourse import bacc
from concourse.tile import TileContext
from concourse.bass_utils import run_bass_kernel_spmd

# Problem shapes (hardcoded per contract)
N, CIN, H, W = 32, 256, 56, 56
COUT, KH, KW = 256, 3, 3
HO, WO = H - KH + 1, W - KW + 1          # 54, 54
NCORES = 8
NPER = N // NCORES                        # images per core
HW = H * W                                # 3136
KT = CIN // 128                           # 2 cin tiles
MT = COUT // 128                          # 2 cout tiles
NU = 4                                    # winograd transform size
JW = WO // 2                              # 27 output col-tiles
NSET = KH * NU                            # 12 weight sets per cin tile
ROWS_C = 18                               # output rows per chunk
NCHUNK = HO // ROWS_C                     # 3
CHUNK = ROWS_C * JW                       # 486 psum cols per u-component
MAGIC = 12582912.0                        # 1.5 * 2**23  (fp32 RNE rounding trick)

_CACHE = {}


def _build_program():
    nc = bacc.Bacc("TRN2", target_bir_lowering=False, debug=False,
                   num_devices=NCORES)
    dt = mybir.dt

    x_d = nc.dram_tensor("x", [NPER, CIN, H, W], dt.int8, kind="ExternalInput")
    wt_d = nc.dram_tensor("wt", [NSET, CIN, COUT], dt.float16,
                          kind="ExternalInput")
    sc_d = nc.dram_tensor("scales", [COUT], dt.float32, kind="ExternalInput")
    bi_d = nc.dram_tensor("bias", [COUT], dt.float32, kind="ExternalInput")
    out_d = nc.dram_tensor("out", [NPER, COUT, HO, WO], dt.int32,
                           kind="ExternalOutput")

    with TileContext(nc) as tc:
        with (
            tc.tile_pool(name="const", bufs=1) as cpool,
            tc.tile_pool(name="xin", bufs=2) as xpool,
            tc.tile_pool(name="vwin", bufs=2) as vpool,
            tc.tile_pool(name="psum", bufs=8, space="PSUM") as ppool,
            tc.tile_pool(name="tmp", bufs=8) as tpool,
            tc.tile_pool(name="outb", bufs=6) as opool,
        ):
            # PE warm-up: tiny matmuls fill the initial DMA wait, flipping
            # the HAM clock gate to 8/8 before the first real matmul.
            wupw = cpool.tile([1, 1], dt.bfloat16)
            nc.vector.memset(wupw[:, :], 1.0)
            wupx = cpool.tile([1, 128], dt.bfloat16)
            nc.vector.memset(wupx[:, :], 1.0)
            # warm-up psum shares the main "ps" ring (8 banks total; each
            # accumulation series must own a whole bank: a start=True matmul
            # resets pending-zero state at 2KB bank granularity)
            wups = ppool.tile([128, CHUNK], dt.float32, name="ps", tag="ps")
            for _ in range(40):
                nc.tensor.matmul(wups[0:1, 0:128], wupw[:, :], wupx[:, :],
                                 start=True, stop=True)

            # pre-transformed fp16 weights: [set, cin, cout] -> lhsT tiles
            wb = cpool.tile([128, KT, NSET, COUT], dt.float16)

            # combined output scale 0.01*scale[o] and folded bias
            sc2 = cpool.tile([128, MT], dt.float32)
            nc.sync.dma_start(out=sc2[:, :],
                              in_=sc_d.rearrange("(m p) -> p m", p=128))
            bi2 = cpool.tile([128, MT], dt.float32)
            nc.sync.dma_start(out=bi2[:, :],
                              in_=bi_d.rearrange("(m p) -> p m", p=128))

            def xtile():
                return xpool.tile([128, KT, HW], dt.int8, name="xi")

            def vtile():
                return vpool.tile([128, KT, H, NU, JW], dt.float16, name="vb")

            def xdma(xi, n, k, r0, r1):
                nc.sync.dma_start(
                    out=xi[:, k, r0 * W:r1 * W],
                    in_=x_d[n, k * 128:(k + 1) * 128].rearrange(
                        "p h w -> p (h w)")[:, r0 * W:r1 * W])

            def vcalc(xi, vb, k, r0, r1):
                # F(2,3) input transform on row range [r0, r1)
                xv = xi[:, k, :].rearrange("p (h j t) -> p h j t", t=2, j=28)
                x0 = xv[:, r0:r1, 0:JW, 0]
                x1 = xv[:, r0:r1, 0:JW, 1]
                x2 = xv[:, r0:r1, 1:JW + 1, 0]
                x3 = xv[:, r0:r1, 1:JW + 1, 1]
                sub = mybir.AluOpType.subtract
                add = mybir.AluOpType.add
                nc.vector.tensor_tensor(vb[:, k, r0:r1, 0, :], x0, x2, sub)
                nc.vector.tensor_tensor(vb[:, k, r0:r1, 1, :], x1, x2, add)
                nc.vector.tensor_tensor(vb[:, k, r0:r1, 2, :], x2, x1, sub)
                nc.vector.tensor_tensor(vb[:, k, r0:r1, 3, :], x1, x3, sub)

            def load_image(n, head=False):
                xi = xtile()
                vb = vtile()
                if head:
                    # interleave weight DMA with the first image's load so
                    # the first matmul fires as early as possible
                    nc.sync.dma_start(
                        out=wb[:, 0, :, :],
                        in_=wt_d[:, 0:128, :].rearrange("t p o -> p t o"))
                for k in range(KT):
                    xdma(xi, n, k, 0, 28)
                    vcalc(xi, vb, k, 0, 28)
                    xdma(xi, n, k, 28, 56)
                    vcalc(xi, vb, k, 28, 56)
                    if head and k == 0:
                        nc.sync.dma_start(
                            out=wb[:, 1, :, :],
                            in_=wt_d[:, 128:256, :].rearrange("t p o -> p t o"))
                return vb

            def emit_chunk(vb, n, m, c):
                # one psum bank per u-series: interleaved start=True matmuls
                # in a shared bank would wipe each other's accumulation
                ps = [ppool.tile([128, CHUNK], dt.float32, name="ps",
                                 tag="ps") for _ in range(NU)]
                for k in range(KT):
                    for dh in range(KH):
                        for u in range(NU):
                            rhs = vb[:, k,
                                     ROWS_C * c + dh:ROWS_C * c + dh + ROWS_C,
                                     u, :]
                            nc.tensor.matmul(
                                ps[u][:, :],
                                wb[:, k, dh * NU + u, m * 128:(m + 1) * 128],
                                rhs, start=(k == 0 and dh == 0),
                                stop=(k == KT - 1 and dh == KH - 1))

                sub = mybir.AluOpType.subtract
                add = mybir.AluOpType.add
                m0, m1, m2, m3 = (ps[u][:, :] for u in range(NU))
                # DVE has one PSUM read port, so stage m1 to SBUF on the
                # (otherwise idle) scalar engine before the m1 +/- m2 ops
                mcp = tpool.tile([128, CHUNK], dt.float32)
                nc.scalar.copy(mcp[:, :], m1)
                t1 = tpool.tile([128, CHUNK], dt.float32)
                t2 = tpool.tile([128, CHUNK], dt.float32)
                nc.vector.tensor_tensor(t1[:, :], mcp[:, :], m2, add)
                nc.vector.tensor_tensor(t2[:, :], mcp[:, :], m2, sub)
                yf = tpool.tile([128, ROWS_C, WO], dt.float32)
                yv = yf[:, :, :].rearrange("p r (j t) -> p r j t", t=2)
                nc.vector.tensor_tensor(
                    yv[:, :, :, 0], m0.rearrange("p (r j) -> p r j", j=JW),
                    t1[:, :].rearrange("p (r j) -> p r j", j=JW), add)
                nc.vector.tensor_tensor(
                    yv[:, :, :, 1], t2[:, :].rearrange("p (r j) -> p r j", j=JW),
                    m3.rearrange("p (r j) -> p r j", j=JW), sub)
                # y = 0.01*scale*Y + bias_f on ACT, then RNE round on DVE
                tmp = tpool.tile([128, ROWS_C * WO], dt.float32)
                nc.scalar.activation(
                    tmp[:, :], yf[:, :, :].rearrange("p r w -> p (r w)"),
                    mybir.ActivationFunctionType.Identity,
                    bias=bi2[:, m:m + 1], scale=sc2[:, m:m + 1])
                ob = opool.tile([128, ROWS_C, WO], dt.int32)
                nc.vector.tensor_scalar(
                    ob[:, :, :],
                    tmp[:, :].rearrange("p (r w) -> p r w", w=WO),
                    MAGIC, MAGIC,
                    mybir.AluOpType.add, mybir.AluOpType.subtract)
                nc.sync.dma_start(
                    out=out_d[n, m * 128:(m + 1) * 128,
                              ROWS_C * c:ROWS_C * (c + 1), :],
                    in_=ob[:, :, :])

            # ---- per-image pipeline ----
            vb = load_image(0, head=True)
            for n in range(NPER):
                vb_next = None
                for m in range(MT):
                    for c in range(NCHUNK):
                        emit_chunk(vb, n, m, c)
                    if m == 0 and n + 1 < NPER:
                        # prefetch next image mid-stream so its DVE transform
                        # overlaps this image's m=1 matmuls
                        vb_next = load_image(n + 1)
                vb = vb_next

    nc.compile()
    return nc


def _prep_weights(w, zp):
    # host-side: g = w - zp (per cout), then G-transform along kw; all
    # values are halves <= 205.5 -> exact in fp16
    g = w.astype(np.float64) - zp.astype(np.float64)[:, None, None, None]
    u = np.empty((KH, NU, CIN, COUT), dtype=np.float64)
    for dh in range(KH):
        gd = g[:, :, dh, :]                       # [cout, cin, kw]
        u[dh, 0] = gd[:, :, 0].T
        u[dh, 1] = ((gd[:, :, 0] + gd[:, :, 1] + gd[:, :, 2]) / 2).T
        u[dh, 2] = ((gd[:, :, 0] - gd[:, :, 1] + gd[:, :, 2]) / 2).T
        u[dh, 3] = gd[:, :, 2].T
    # [set = dh*NU+u, cin, cout]
    return np.ascontiguousarray(
        u.reshape(NSET, CIN, COUT).astype(np.float16))


def _prep_scalars(w, zp, scales, bias):
    g64 = (w.astype(np.float64)
           - zp.astype(np.float64)[:, None, None, None]).sum(axis=(1, 2, 3))
    sc = (0.01 * scales.astype(np.float64)).astype(np.float32)
    bi = (bias.astype(np.float64)
          - 0.07 * scales.astype(np.float64) * g64).astype(np.float32)
    return sc, bi


def kernel(**inputs) -> np.ndarray:
    x = np.ascontiguousarray(np.asarray(inputs["inputVec"], dtype=np.int8))
    w = np.asarray(inputs["weight"], dtype=np.int8)
    scales = np.asarray(inputs["scales"], dtype=np.float32)
    zp = np.asarray(inputs["zeropoints"], dtype=np.int32)
    bias = np.asarray(inputs["bias"], dtype=np.float32)
    assert x.shape == (N, CIN, H, W) and w.shape == (COUT, CIN, KH, KW)

    wt = _prep_weights(w, zp)
    sc, bi = _prep_scalars(w, zp, scales, bias)

    if "nc" not in _CACHE:
        _CACHE["nc"] = _build_program()
    nc = _CACHE["nc"]

    in_maps = [
        {"x": x[c * NPER:(c + 1) * NPER], "wt": wt, "scales": sc, "bias": bi}
        for c in range(NCORES)
    ]
    res = run_bass_kernel_spmd(nc, in_maps, list(range(NCORES)))
    out = np.concatenate([res.results[c]["out"] for c in range(NCORES)], axis=0)
    return out


# revision 14
# speedup vs baseline: 1.2554x; 1.0330x over previous
"""Trainium2 Bass kernel for quantized int8 per-channel Conv2d.

Reference semantics (fp32):
  x_f = (x_int8 - 7) * 0.01                      # per-tensor dequant
  w_f = (w_int8 - zp[cout]) * scale[cout]        # per-channel dequant
  y   = round(conv2d_valid(x_f, w_f) + bias[cout])  -> int32

Winograd F(2,3) along W (direct conv along H), exact in fp16:
  Per 2 output cols j..j+1 and tap row dh, with g = w - zp (host-side):
    V0 = x0-x2, V1 = x1+x2, V2 = x2-x1, V3 = x1-x3   (ints <= 270: fp16-exact)
    U0 = g0, U1 = (g0+g1+g2)/2, U2 = (g0-g1+g2)/2, U3 = g2  (halves: fp16-exact)
    y0 = sum m0+m1+m2,  y1 = sum m1-m2-m3,  m_u = V_u . U_u  (over cin, dh)
  12 column-streams per output pixel per m-tile instead of 18 -> 0.667x PE time.
  The x-7 offset cancels in V0/V2/V3 and contributes a per-cout constant via
  V1 (since 2*sum U1 = sum g exactly), folded into bias on the host:
    bias_f = bias - 0.07*scale*sum(w - zp).
  Products are half-integers < 2^17, accumulated exactly in fp32 PSUM; final
  affine + magic-number round matches jnp.round to ~1e-5 rel (half-ULP ties).

Sharding: data-parallel over batch N=32 across 8 cores (4 images each);
weights/scales/bias replicated.
"""

import numpy as np

import concourse.bass as bass
import concourse.mybir as mybir
from concourse import bacc
from concourse.tile import TileContext
from concourse.bass_utils import run_bass_kernel_spmd

# Problem shapes (hardcoded per contract)
N, CIN, H, W = 32, 256, 56, 56
COUT, KH, KW = 256, 3, 3
HO, WO = H - KH + 1, W - KW + 1          # 54, 54
NCORES = 8
NPER = N // NCORES                        # images per core
HW = H * W                                # 3136
KT = CIN // 128                           # 2 cin tiles
MT = COUT // 128                          # 2 cout tiles
NU = 4                                    # winograd transform size
JW = WO // 2                              # 27 output col-tiles
NSET = KH * NU                            # 12 weight sets per cin tile
ROWS_C = 18                               # output rows per chunk
NCHUNK = HO // ROWS_C                     # 3
CHUNK = ROWS_C * JW                       # 486 psum cols per u-component
MAGIC = 12582912.0                        # 1.5 * 2**23  (fp32 RNE rounding trick)

_CACHE = {}


def _build_program():
    nc = bacc.Bacc("TRN2", target_bir_lowering=False, debug=False,
                   num_devices=NCORES)
    dt = mybir.dt

    x_d = nc.dram_tensor("x", [NPER, CIN, H, W], dt.int8, kind="ExternalInput")
    wt_d = nc.dram_tensor("wt", [NSET, CIN, COUT], dt.float16,
                          kind="ExternalInput")
    sc_d = nc.dram_tensor("scales", [COUT], dt.float32, kind="ExternalInput")
    bi_d = nc.dram_tensor("bias", [COUT], dt.float32, kind="ExternalInput")
    out_d = nc.dram_tensor("out", [NPER, COUT, HO, WO], dt.int32,
                           kind="ExternalOutput")

    with TileContext(nc) as tc:
        with (
            tc.tile_pool(name="const", bufs=1) as cpool,
            tc.tile_pool(name="xin", bufs=2) as xpool,
            tc.tile_pool(name="vwin", bufs=2) as vpool,
            tc.tile_pool(name="psum", bufs=8, space="PSUM") as ppool,
            tc.tile_pool(name="tmp", bufs=8) as tpool,
            tc.tile_pool(name="outb", bufs=6) as opool,
        ):
            # PE warm-up: tiny matmuls fill the initial DMA wait, flipping
            # the HAM clock gate to 8/8 before the first real matmul.
            wupw = cpool.tile([1, 1], dt.bfloat16)
            nc.vector.memset(wupw[:, :], 1.0)
            wupx = cpool.tile([1, 128], dt.bfloat16)
            nc.vector.memset(wupx[:, :], 1.0)
            # warm-up psum shares the main "ps" ring (8 banks total; each
            # accumulation series must own a whole bank: a start=True matmul
            # resets pending-zero state at 2KB bank granularity)
            wups = ppool.tile([128, CHUNK], dt.float32, name="ps", tag="ps")
            for _ in range(40):
                nc.tensor.matmul(wups[0:1, 0:128], wupw[:, :], wupx[:, :],
                                 start=True, stop=True)

            # pre-transformed fp16 weights: [set, cin, cout] -> lhsT tiles
            wb = cpool.tile([128, KT, NSET, COUT], dt.float16)

            # combined output scale 0.01*scale[o] and folded bias (DMA'd
            # inside load_image(0) after the critical first x/w pieces)
            sc2 = cpool.tile([128, MT], dt.float32)
            bi2 = cpool.tile([128, MT], dt.float32)

            def xtile():
                return xpool.tile([128, KT, HW], dt.int8, name="xi")

            def vtile():
                return vpool.tile([128, KT, H, NU, JW], dt.float16, name="vb")

            def xdma(xi, n, k, r0, r1):
                nc.sync.dma_start(
                    out=xi[:, k, r0 * W:r1 * W],
                    in_=x_d[n, k * 128:(k + 1) * 128].rearrange(
                        "p h w -> p (h w)")[:, r0 * W:r1 * W])

            def vcalc(xi, vb, k, r0, r1):
                # F(2,3) input transform on row range [r0, r1)
                xv = xi[:, k, :].rearrange("p (h j t) -> p h j t", t=2, j=28)
                x0 = xv[:, r0:r1, 0:JW, 0]
                x1 = xv[:, r0:r1, 0:JW, 1]
                x2 = xv[:, r0:r1, 1:JW + 1, 0]
                x3 = xv[:, r0:r1, 1:JW + 1, 1]
                sub = mybir.AluOpType.subtract
                add = mybir.AluOpType.add
                nc.vector.tensor_tensor(vb[:, k, r0:r1, 0, :], x0, x2, sub)
                nc.vector.tensor_tensor(vb[:, k, r0:r1, 1, :], x1, x2, add)
                nc.vector.tensor_tensor(vb[:, k, r0:r1, 2, :], x2, x1, sub)
                nc.vector.tensor_tensor(vb[:, k, r0:r1, 3, :], x1, x3, sub)

            def load_image(n, head=False):
                xi = xtile()
                vb = vtile()
                if head:
                    # critical path to the first matmul: x(k0) rows then the
                    # k0 weights; everything else is interleaved behind them
                    xdma(xi, n, 0, 0, 28)
                    nc.sync.dma_start(
                        out=wb[:, 0, :, :],
                        in_=wt_d[:, 0:128, :].rearrange("t p o -> p t o"))
                    vcalc(xi, vb, 0, 0, 28)
                    xdma(xi, n, 0, 28, 56)
                    vcalc(xi, vb, 0, 28, 56)
                    xdma(xi, n, 1, 0, 28)
                    vcalc(xi, vb, 1, 0, 28)
                    nc.sync.dma_start(
                        out=wb[:, 1, :, :],
                        in_=wt_d[:, 128:256, :].rearrange("t p o -> p t o"))
                    xdma(xi, n, 1, 28, 56)
                    vcalc(xi, vb, 1, 28, 56)
                    nc.sync.dma_start(out=sc2[:, :],
                                      in_=sc_d.rearrange("(m p) -> p m", p=128))
                    nc.sync.dma_start(out=bi2[:, :],
                                      in_=bi_d.rearrange("(m p) -> p m", p=128))
                else:
                    for k in range(KT):
                        xdma(xi, n, k, 0, 56)
                        vcalc(xi, vb, k, 0, 56)
                return vb

            def emit_chunk(vb, n, m, c, r0=0, nr=ROWS_C):
                # one psum bank per u-series: interleaved start=True matmuls
                # in a shared bank would wipe each other's accumulation.
                # Tiles are always full-bank; sub-chunks (nr < ROWS_C, used to
                # shorten the kernel tail) just use the first nr*JW columns.
                nj = nr * JW
                base = ROWS_C * c + r0
                ps = [ppool.tile([128, CHUNK], dt.float32, name="ps",
                                 tag="ps") for _ in range(NU)]
                for k in range(KT):
                    for dh in range(KH):
                        for u in range(NU):
                            rhs = vb[:, k, base + dh:base + dh + nr, u, :]
                            nc.tensor.matmul(
                                ps[u][:, 0:nj],
                                wb[:, k, dh * NU + u, m * 128:(m + 1) * 128],
                                rhs, start=(k == 0 and dh == 0),
                                stop=(k == KT - 1 and dh == KH - 1))

                sub = mybir.AluOpType.subtract
                add = mybir.AluOpType.add
                m0, m1, m2, m3 = (ps[u][:, 0:nj] for u in range(NU))
                # DVE has one PSUM read port, so stage m1 to SBUF on the
                # (otherwise idle) scalar engine before the m1 +/- m2 ops
                mcp = tpool.tile([128, CHUNK], dt.float32)
                nc.scalar.copy(mcp[:, 0:nj], m1)
                t1 = tpool.tile([128, CHUNK], dt.float32)
                t2 = tpool.tile([128, CHUNK], dt.float32)
                nc.vector.tensor_tensor(t1[:, 0:nj], mcp[:, 0:nj], m2, add)
                nc.vector.tensor_tensor(t2[:, 0:nj], mcp[:, 0:nj], m2, sub)
                yf = tpool.tile([128, ROWS_C, WO], dt.float32)
                yv = yf[:, 0:nr, :].rearrange("p r (j t) -> p r j t", t=2)
                nc.vector.tensor_tensor(
                    yv[:, :, :, 0], m0.rearrange("p (r j) -> p r j", j=JW),
                    t1[:, 0:nj].rearrange("p (r j) -> p r j", j=JW), add)
                nc.vector.tensor_tensor(
                    yv[:, :, :, 1], t2[:, 0:nj].rearrange("p (r j) -> p r j", j=JW),
                    m3.rearrange("p (r j) -> p r j", j=JW), sub)
                # y = round(0.01*scale*Y + bias_f) -> int32, all in one ACT op
                # (the fp32->int32 output conversion rounds to nearest even)
                ob = opool.tile([128, ROWS_C, WO], dt.int32)
                nc.scalar.activation(
                    ob[:, 0:nr, :].rearrange("p r w -> p (r w)"),
                    yf[:, 0:nr, :].rearrange("p r w -> p (r w)"),
                    mybir.ActivationFunctionType.Identity,
                    bias=bi2[:, m:m + 1], scale=sc2[:, m:m + 1])
                nc.sync.dma_start(
                    out=out_d[n, m * 128:(m + 1) * 128, base:base + nr, :],
                    in_=ob[:, 0:nr, :])

            # ---- per-image pipeline ----
            vb = load_image(0, head=True)
            for n in range(NPER):
                vb_next = None
                for m in range(MT):
                    for c in range(NCHUNK):
                        if n == NPER - 1 and m == MT - 1 and c == NCHUNK - 1:
                            # split the final chunk so the first half's
                            # epilogue overlaps the second half's matmuls
                            emit_chunk(vb, n, m, c, 0, ROWS_C // 2)
                            emit_chunk(vb, n, m, c, ROWS_C // 2, ROWS_C // 2)
                        else:
                            emit_chunk(vb, n, m, c)
                        if m == 0 and c == 0 and n + 1 < NPER:
                            # prefetch next image right after the first chunk
                            # so its DMA + DVE transform run early enough to
                            # never stall the PE at the image boundary
                            vb_next = load_image(n + 1)
                vb = vb_next

    nc.compile()
    return nc


def _prep_weights(w, zp):
    # host-side: g = w - zp (per cout), then G-transform along kw; all
    # values are halves <= 205.5 -> exact in fp16
    g = w.astype(np.float64) - zp.astype(np.float64)[:, None, None, None]
    u = np.empty((KH, NU, CIN, COUT), dtype=np.float64)
    for dh in range(KH):
        gd = g[:, :, dh, :]                       # [cout, cin, kw]
        u[dh, 0] = gd[:, :, 0].T
        u[dh, 1] = ((gd[:, :, 0] + gd[:, :, 1] + gd[:, :, 2]) / 2).T
        u[dh, 2] = ((gd[:, :, 0] - gd[:, :, 1] + gd[:, :, 2]) / 2).T
        u[dh, 3] = gd[:, :, 2].T
    # [set = dh*NU+u, cin, cout]
    return np.ascontiguousarray(
        u.reshape(NSET, CIN, COUT).astype(np.float16))


def _prep_scalars(w, zp, scales, bias):
    g64 = (w.astype(np.float64)
           - zp.astype(np.float64)[:, None, None, None]).sum(axis=(1, 2, 3))
    sc = (0.01 * scales.astype(np.float64)).astype(np.float32)
    bi = (bias.astype(np.float64)
          - 0.07 * scales.astype(np.float64) * g64).astype(np.float32)
    return sc, bi


def kernel(**inputs) -> np.ndarray:
    x = np.ascontiguousarray(np.asarray(inputs["inputVec"], dtype=np.int8))
    w = np.asarray(inputs["weight"], dtype=np.int8)
    scales = np.asarray(inputs["scales"], dtype=np.float32)
    zp = np.asarray(inputs["zeropoints"], dtype=np.int32)
    bias = np.asarray(inputs["bias"], dtype=np.float32)
    assert x.shape == (N, CIN, H, W) and w.shape == (COUT, CIN, KH, KW)

    wt = _prep_weights(w, zp)
    sc, bi = _prep_scalars(w, zp, scales, bias)

    if "nc" not in _CACHE:
        _CACHE["nc"] = _build_program()
    nc = _CACHE["nc"]

    in_maps = [
        {"x": x[c * NPER:(c + 1) * NPER], "wt": wt, "scales": sc, "bias": bi}
        for c in range(NCORES)
    ]
    res = run_bass_kernel_spmd(nc, in_maps, list(range(NCORES)))
    out = np.concatenate([res.results[c]["out"] for c in range(NCORES)], axis=0)
    return out
